# revision 38
# baseline (speedup 1.0000x reference)
"""TRN2 Bass kernel for nn_BSquareModelCombined (spiking MLP, LIF neurons).

Strategy
--------
The reference scans over T=100 steps, but the GEMMs are state-independent:
  h1 = x_t @ W1^T  for all t  -> one big GEMM over R = T*B_loc rows
  LIF scan (elementwise) -> spikes s1
  h2 = s1 @ W2^T   -> one big GEMM;  LIF scan -> s2
  h3 = s2 @ W3^T   -> small GEMM; output-layer scan + voting on host.

Data-parallel over batch: 8 cores x 4 batch rows. On-chip layout is
feature-major ("transposed"): activations are [D, R] with r = t*4+b, so the
GEMM moving operand is an activation tile [128, R=400] and the stationary
operand is a weight tile [128, 128].

Precision (the LIF thresholds make the network chaotic; host sims show the
final votes need ~16-bit weight fidelity in W1/W2 and ~24-bit x, while W3
tolerates 12-bit):
 - GEMM1: 3 passes in float32r (PE truncates operands to a 12-bit
   significand; a 12-bit hi/lo split of x and W1 is exactly representable,
   so xhi*Whi + xlo*Whi + xhi*Wlo is fp32-exact minus a 2^-24 term).
   All 3 passes accumulate into ONE PSUM group per m-tile (single
   eviction), and the last K-tile (only 8 real features of 2312) packs
   all three pass contributions into one matmul along spare partitions.
 - GEMM2: spikes are sign-encoded (g = sign(m) = 2s-1), stored as fp8.
   h = g @ (W/2)^T + rowsum(W/2), with W/2 decomposed into NCH=3 fp8e4m3
   chunks at one global scale S1=2^14 (clip+RNE greedy residual split;
   ~13-bit effective fidelity, host sims show final votes at ~1.3e-2 rel
   vs the 2e-2 gate). Both operands fp8 enables DoubleRow perf mode: one
   matmul contracts TWO k-tiles at bf16-rate (2x fp8 throughput measured
   on HW), so 3 chunks cost 69 matmul-slots/m-tile vs 90 for bf16 hi/lo.
   All chunks share one PSUM group; eviction applies 1/S1 and the bias
   (rowsum correction over the reconstructed chunks, fp64 on host).
 - GEMM3: spikes fp8 moving x bf16 stationary hi/lo (legal — only 32-bit
   dtypes must match).

LIF scan: one fused custom DVE op per step computes
   m_t = beta*m + h_t - (m > 0)   (reset recomputed from sign, not stored)
and the Scalar engine's Sign activation emits g_t = sign(m_t) off the
critical path. A serial chain steps at ~320ns (op + result-commit), so the
last group of each scan — whose input finalizes only when the producing
GEMM ends — trails by ~32us. Mitigations: small last groups, JIT K-order
in the consuming GEMM, and interleaved PSUM accumulation groups for
GEMM2's first two m-tiles to widen the overlap window.
"""
import sys

sys.path.insert(0, "/opt/trn_rl_repo")
sys.path.insert(0, "/root/.axon_site")

import numpy as np
import ml_dtypes

import concourse.bass as bass  # noqa: F401
import concourse.tile as tile
from concourse import bacc, mybir
from concourse import dve_ops
from concourse.dve_spec import Spec, Src0, Src1, C0, Zero, lower as dve_lower
from concourse.dve_uop import DveOpSpec
from concourse.bass_utils import run_bass_kernel_spmd

F32 = mybir.dt.float32
F32R = mybir.dt.float32r
F16 = mybir.dt.float16
BF16 = mybir.dt.bfloat16
FP8 = mybir.dt.float8e4

B, T_FULL, DIN, DH, DOUT = 32, 100, 2312, 5760, 90
NCORES = 8
BL = B // NCORES            # batch rows per core
KP = 19                     # D_in tiles after padding 2312 -> 2432
KF = 18                     # full 128-deep K tiles; tile 18 holds 8 feats
DINP = KP * 128
MT = DH // 128              # 45 feature tiles
BETA, THRESH = 0.9, 1.0
NUM_CLASSES, TRI_NUM = 10, 45
NCH = 3                     # fp8 chunks of W2/2 (4 = extra-safe fallback)
S1E = 13                    # chunk scale exponent: stored = fp8(W/2 * 2^S1E)
FP8_MAX = 240.0             # mybir float8e4 is IEEE e4m3: exp 1111 = inf/nan
DR = mybir.MatmulPerfMode.DoubleRow

_nc_cache = {}
_prep_cache = {}
DEBUG_TAPS = False          # extra DRAM outputs (g1, full h2) for debugging


def _register_lif_op():
    """Fused LIF membrane update: out = s0*in0 + in1 - (in0 > 0)."""
    name = "LIF_STEP_ANT"
    for o in dve_ops.OPS:
        if o.name == name:
            return o
    spec = Spec(
        body=(Src0 * C0) + Src1 - (Src0 > Zero),
        reference=lambda in0, in1, s0, s1, imm2: in0.astype(np.float32) * s0
        + in1.reshape(in0.shape)
        - (in0 > 0).astype(np.float32),
    )
    row = max(dve_ops._SUB_OPCODE_FOR_NAME.values()) + 1
    shas = {}
    for ver in ("v3", "v4"):
        uops = dve_lower(spec, ver=ver)
        shas[ver] = DveOpSpec(name=name, opcode=row, uops=uops, rd1_en=True).sha(ver)
    op = dve_ops.DveOp(name, spec, subdim=False, uops_sha=shas)
    dve_ops.OPS.append(op)
    dve_ops.CUSTOM_DVE_SPECS[name] = spec
    dve_ops._SUB_OPCODE_FOR_NAME[name] = row
    return op


LIF_OP = _register_lif_op()

# scan group layouts: (start_tile, n_tiles) lists. The last group is small
# so its 100-step serial DVE chain (the only part that can't hide under the
# producing GEMM) ends sooner; the consuming GEMM orders that group's
# K-tiles last (JIT) to hide the remaining chain latency.
SCAN1_GROUPS = [(0, 15), (15, 15), (30, 10), (40, 5)]
# scan2 covers only tiles 0..41 on-chip: tiles 42-44 evict LAST from GEMM2
# and their scan + GEMM3 contribution moves to the host (h2t output), so no
# scan chain ever trails the last GEMM — GEMM3 runs stall-free.
SCAN2_GROUPS = [(0, 9), (9, 9), (18, 9), (27, 9), (36, 6)]
MT3 = 42                    # feature tiles contracted on-chip in GEMM3
HOST_TILES = 3              # h2 tiles 42-44 handled on host


def _build(T):
    """Build + compile the per-core program (same program on all 8 cores)."""
    R = T * BL
    nc = bacc.Bacc(None, target_bir_lowering=False)

    # x split into per-K-tile chunks so the first matmul starts early
    # GEMM1 fully f16 (walrus only allows f32r paired with f32r): x and W1
    # each split as f16 hi + 2^12-scaled f16 lo (~22-bit effective). The
    # hi*hi pass accumulates in PSUM group A; both refinement passes
    # (xl_s@wh and xh@wl_s) carry the same 2^12 scale and share group B,
    # descaled at eviction. Halves both x and W1 DMA vs f32r.
    xhi_d = nc.dram_tensor("xhi", [128, KP, R], F16, kind="ExternalInput")
    xlo_d = nc.dram_tensor("xlo", [128, KP, R], F16, kind="ExternalInput")
    w1hi_d = nc.dram_tensor("w1hi", [MT, 128, KP, 128], F16, kind="ExternalInput")
    w1lo_d = nc.dram_tensor("w1lo", [MT, 128, KP, 128], F16, kind="ExternalInput")
    G1L = SCAN1_GROUPS[-1][0]   # deferral boundary (40, even: DR pairs align)
    # late tiles padded to an even 6 k-tiles (k45 = zeros, paired with the
    # zero-spike g1 tile 45) so every GEMM2 matmul is DoubleRow — a
    # DR<->normal perf-mode switch costs a ~310ns PE bubble.
    MTP = MT + 1
    w2e_d = nc.dram_tensor("w2e", [MT, NCH, 128, G1L, 128], FP8,
                           kind="ExternalInput")
    w2l_d = nc.dram_tensor("w2l", [MT, NCH, 128, MTP - G1L, 128], FP8,
                           kind="ExternalInput")
    w3_d = nc.dram_tensor("w3", [128, 2, MT, DOUT], BF16, kind="ExternalInput")
    b12_d = nc.dram_tensor("b12", [128, 2 * MT], F32, kind="ExternalInput")
    h3o = nc.dram_tensor("h3", [DOUT, R], F32, kind="ExternalOutput")
    h2t_d = nc.dram_tensor("h2t", [128, HOST_TILES, R], F32,
                           kind="ExternalOutput")

    add = mybir.AluOpType.add
    mult = mybir.AluOpType.mult

    with tile.TileContext(nc) as tc:
        with (
            tc.tile_pool(name="acts", bufs=1) as acts,
            tc.tile_pool(name="wpool", bufs=3) as wpool,
            tc.tile_pool(name="small", bufs=1) as small,
            tc.tile_pool(name="psum", bufs=4, space="PSUM") as pp,
        ):
            xhi = acts.tile([128, KP, R], F16, tag="xhi")
            xlo = acts.tile([128, KP, R], F16, tag="s_or_xlo")
            hsb = acts.tile([128, MT, R], F32, tag="h")
            b12sb = small.tile([128, 2 * MT], F32)
            # Rotating scan-state buffers shared by all groups of both
            # scans; each group uses its own disjoint column slice.
            # 6 buffers (not 3): the Scalar sign of step t reads buffer
            # (t+1)%NB, and with a short rotation the step t+NB-1 DVE write
            # would wait on that cross-engine read (~300ns sign) every
            # rotation, throttling the serial scan chain.
            NB = 5
            sts = [small.tile([128, MT, BL], F32, name=f"st{i}")
                   for i in range(NB)]

            # ---- GEMM1: group A = xh@wh (19 mm); group B (2^12 scale) =
            # xh@wl_s (18) + xl_s@wh (18) + packed k18 tail (1). Pass order
            # puts the xh-moving passes first so m0 runs 37 of its 56
            # matmuls while the xl stream is still landing.
            with nc.named_scope("gemm1"):
                for m in range(MT):
                    whi = wpool.tile([128, KP, 128], F16, tag="w")
                    wlo = wpool.tile([128, KP, 128], F16, tag="w")
                    if m == 0:
                        # startup: the first matmul needs only whi k0..3
                        # plus xhi k0, not the full whi tile. wlo and b12
                        # ride ahead of the x streams.
                        nc.sync.dma_start(out=whi[:, 0:4, :],
                                          in_=w1hi_d.ap()[m][:, 0:4])
                        nc.sync.dma_start(out=whi[:, 4:KP, :],
                                          in_=w1hi_d.ap()[m][:, 4:KP])
                        nc.sync.dma_start(out=b12sb[:], in_=b12_d.ap())
                        nc.sync.dma_start(out=wlo[:], in_=w1lo_d.ap()[m])
                        for k in range(KP):
                            nc.sync.dma_start(out=xhi[:, k, :],
                                              in_=xhi_d.ap()[:, k])
                        for k in range(KP):
                            nc.sync.dma_start(out=xlo[:, k, :],
                                              in_=xlo_d.ap()[:, k])
                    else:
                        nc.sync.dma_start(out=whi[:], in_=w1hi_d.ap()[m])
                        nc.sync.dma_start(out=wlo[:], in_=w1lo_d.ap()[m])
                    psa = pp.tile([128, R], F32, tag="ps", name=f"psa_{m}")
                    psb = pp.tile([128, R], F32, tag="ps", name=f"psb_{m}")
                    # pass 1: xh @ wh
                    for k in range(KP):
                        nc.tensor.matmul(
                            psa[:], whi[:, k, :], xhi[:, k, :],
                            start=(k == 0), stop=(k == KP - 1),
                        )
                    # pass 2: xh @ wl_s (group B)
                    for k in range(KF):
                        nc.tensor.matmul(
                            psb[:], wlo[:, k, :], xhi[:, k, :],
                            start=(k == 0), stop=False,
                        )
                    # pass 3: xl_s @ wh (group B)
                    for k in range(KF):
                        nc.tensor.matmul(
                            psb[:], whi[:, k, :], xlo[:, k, :],
                            start=False, stop=False,
                        )
                    # k18 tail, packed: moving [xl8_s; xh8] x [wh8; wl8_s]
                    nc.tensor.matmul(
                        psb[:], wlo[:, KF, :], xlo[:, KF, :],
                        start=False, stop=True,
                    )
                    nc.vector.tensor_scalar(
                        hsb[:, m, :], psb[:], float(2.0 ** -12),
                        b12sb[:, m : m + 1], mult, add
                    )
                    nc.vector.tensor_tensor(
                        hsb[:, m, :], hsb[:, m, :], psa[:], add
                    )

            # spikes, sign-encoded (+1 spike / -1 no spike), fp8 (exact).
            # g1 gets its OWN region (no alias with x): its signs must run
            # during GEMM1's tail, while xhi/xlo are still being read.
            # g1 tile 45 is the zero pad partnering w2l's k45 zero weights.
            g1sb = acts.tile([128, MTP, R], FP8, tag="g1")
            nc.vector.memset(g1sb[:, MT, :], 0.0)
            g2sb = acts.tile([128, MT, R], FP8, tag="xhi")

            def lif_scan(scope, gsb, groups):
                # chunk-group scans: group g only depends on its own feature
                # tiles, so it starts as soon as the producing GEMM has
                # evicted those tiles and hides under the GEMM. The LAST
                # group's chain cannot hide (its inputs finalize only when
                # the GEMM ends), and a single chain steps at ~320ns (op +
                # result-commit latency). Splitting it into BL independent
                # per-batch-column chains interleaves 4 ops per step, hiding
                # the commit latency — identical math, ~2x lower latency.
                with nc.named_scope(scope):
                    for g, (c0, n) in enumerate(groups):
                        c1 = c0 + n
                        nc.vector.memset(sts[0][:, c0:c1, :], -1.0)
                        for t in range(T):
                            hsl = hsb[:, c0:c1, BL * t : BL * (t + 1)]
                            gsl = gsb[:, c0:c1, BL * t : BL * (t + 1)]
                            src = sts[t % NB][:, c0:c1, :]
                            dst = sts[(t + 1) % NB][:, c0:c1, :]
                            nc.vector._custom_dve(
                                LIF_OP, out=dst, in0=src, in1=hsl, s0=BETA
                            )
                            nc.scalar.sign(gsl, dst)

            lif_scan("scan1", g1sb, SCAN1_GROUPS)

            # ---- GEMM2: h2 = g1 @ (W2/2)^T + bias'' (NCH fp8 chunks) ----
            # The last scan1 group's (spike tiles 40-44) serial chain only
            # starts after GEMM1's final eviction. To hide its ~32us, the
            # first HEAD_M output tiles emit only k<G1L (all chunks) as
            # open PSUM groups, then their k>=G1L tail runs. Weights are
            # split early/late at G1L so a deferred m-tile pins only its
            # tiny late tiles. Output tiles 42-44 (host-bound) come last.
            M_ORDER = list(range(MT3)) + list(range(MT3, MT))
            HEAD_M = 4
            TOTAL_MM = NCH * (G1L // 2 + (MTP - G1L) // 2)

            def g2_weights(m):
                wes, wls = [], []
                for c in range(NCH):
                    we = wpool.tile([128, G1L, 128], FP8, tag="w2e",
                                    bufs=6, name=f"w2e_{m}_{c}")
                    nc.sync.dma_start(out=we[:], in_=w2e_d.ap()[m][c])
                    wes.append(we)
                for c in range(NCH):
                    wl = wpool.tile([128, MTP - G1L, 128], FP8, tag="w2l",
                                    bufs=3 * HEAD_M, name=f"w2l_{m}_{c}")
                    nc.sync.dma_start(out=wl[:], in_=w2l_d.ap()[m][c])
                    wls.append(wl)
                return wes, wls

            def g2_evict(m, ps):
                nc.vector.tensor_scalar(
                    hsb[:, m, :], ps[:], float(2.0 ** -S1E),
                    b12sb[:, MT + m : MT + m + 1], mult, add
                )

            with nc.named_scope("gemm2"):
                st = {}  # m -> [ps, wes, wls, nmm]

                def emit(m, lo, hi):
                    # hi == MT means "through the padded tile 45"
                    hi_pad = MTP if hi == MT else hi
                    ps, wes, wls, nmm = st[m]
                    for c in range(NCH):
                        for k in range(lo, hi_pad, 2):
                            w_ap = (wes[c][:, k : k + 2, :] if k < G1L
                                    else wls[c][:, k - G1L : k - G1L + 2, :])
                            nc.tensor.matmul(
                                ps[:], w_ap, g1sb[:, k : k + 2, :],
                                start=(nmm == 0),
                                stop=(nmm == TOTAL_MM - 1),
                                perf_mode=DR,
                            )
                            nmm += 1
                    st[m][3] = nmm

                for m in M_ORDER[:HEAD_M]:
                    wes, wls = g2_weights(m)
                    ps = pp.tile([128, R], F32, tag="ps", name=f"ps_g2_{m}")
                    st[m] = [ps, wes, wls, 0]
                    emit(m, 0, G1L)
                for m in M_ORDER[:HEAD_M]:
                    emit(m, G1L, MT)
                    g2_evict(m, st[m][0])
                # steady state
                w3sb = None
                for i, m in enumerate(M_ORDER[HEAD_M:]):
                    if i == len(M_ORDER) - HEAD_M - 3:
                        # prefetch W3 (hi+lo halves in one tile, living in
                        # the long-dead xlo region) so GEMM3 starts clean
                        w3sb = acts.tile([128, 2, MT, DOUT], BF16,
                                         tag="s_or_xlo", name="w3sb")
                        nc.sync.dma_start(out=w3sb[:], in_=w3_d.ap())
                    wes, wls = g2_weights(m)
                    ps = pp.tile([128, R], F32, tag="ps", name=f"ps_g2_{m}")
                    st[m] = [ps, wes, wls, 0]
                    emit(m, 0, MT)
                    g2_evict(m, ps)
                # ship host-bound h2 tiles (42-44) for the host-side tail
                nc.sync.dma_start(out=h2t_d.ap(), in_=hsb[:, MT3:MT, :])

            if DEBUG_TAPS:
                h2f_d = nc.dram_tensor("h2f", [128, MT, R], F32,
                                       kind="ExternalOutput")
                nc.sync.dma_start(out=h2f_d.ap(), in_=hsb[:])
                g1o_d = nc.dram_tensor("g1o", [128, MT, R], FP8,
                                       kind="ExternalOutput")
                nc.sync.dma_start(out=g1o_d.ap(), in_=g1sb[:, :MT, :])

            lif_scan("scan2", g2sb, SCAN2_GROUPS)

            if DEBUG_TAPS:
                g2o_d = nc.dram_tensor("g2o", [128, MT3, R], FP8,
                                       kind="ExternalOutput")
                nc.sync.dma_start(out=g2o_d.ap(), in_=g2sb[:, :MT3, :])

            # ---- GEMM3: h3 = g2 @ (W3/2)^T (hi/lo bf16), out [90, R] ----
            # only tiles 0..41: every scan2 group finishes before GEMM2
            # does (the last group's tiles evict 3 m-tiles early), so these
            # 84 matmuls run without any scan stall. Tiles 42-44 are summed
            # on the host from the h2t output.
            with nc.named_scope("gemm3"):
                ps3 = pp.tile([DOUT, R], F32, tag="ps3")
                nmm = 0
                for k in range(MT3):
                    for h in (0, 1):
                        nc.tensor.matmul(
                            ps3[:], w3sb[:, h, k, :], g2sb[:, k, :],
                            start=(nmm == 0), stop=(nmm == 2 * MT3 - 1),
                        )
                        nmm += 1
                h3sb = small.tile([DOUT, R], F32, tag="h3sb")
                nc.vector.tensor_copy(h3sb[:], ps3[:])
                nc.sync.dma_start(out=h3o.ap(), in_=h3sb[:])

    nc.compile()
    return nc


def _bf(a):
    return a.astype(ml_dtypes.bfloat16)


def _round12(a):
    """Round fp32 to 12-bit significand (11 explicit mantissa bits), RNE —
    the f32r PE operand grid; representable values pass the PE unchanged."""
    u = np.ascontiguousarray(a, np.float32).view(np.uint32)
    u = (u + 0x7FF + ((u >> 12) & 1)) & np.uint32(0xFFFFF000)
    return u.view(np.float32)


def _prep_weights(fc1_w, fc1_b, fc2_w, fc2_b, fco_w):
    key = (fc1_w.ctypes.data, fc2_w.ctypes.data, fco_w.ctypes.data)
    if key in _prep_cache:
        return _prep_cache[key]
    # GEMM1: f16 hi + 2^12-scaled f16 lo split of W1^T tiles (~22-bit W1)
    W1p = np.zeros((DH, DINP), np.float32)
    W1p[:, :DIN] = fc1_w
    W1t = np.ascontiguousarray(
        W1p.reshape(MT, 128, KP, 128).transpose(0, 3, 2, 1)
    )  # [m, p, k, q] = W1[m*128+q, k*128+p]
    w1hi = W1t.astype(np.float16)
    w1lo = ((W1t - w1hi.astype(np.float32)) * 4096.0).astype(np.float16)
    # K-tile 18 (8 real features): group A's matmul uses w1hi k18 p0-7 as
    # is. Group B's single packed k18 matmul uses the w1lo tile laid out
    # [wh8; wl8_s] against moving [xl8_s; xh8].
    lo18 = w1lo[:, 0:8, KF, :].copy()
    w1lo[:, 0:8, KF, :] = w1hi[:, 0:8, KF, :]
    w1lo[:, 8:16, KF, :] = lo18
    w1hi = np.ascontiguousarray(w1hi)
    w1lo = np.ascontiguousarray(w1lo)
    # GEMM2: sign-encoded spikes -> weights W2/2, NCH fp8e4m3 chunks at one
    # global scale 2^S1E (greedy clip+RNE residual split; stored = r*2^S1E)
    W2t = fc2_w.reshape(MT, 128, MT, 128).transpose(0, 3, 2, 1) * 0.5
    S1 = float(2.0 ** S1E)
    resid = W2t.astype(np.float64)
    w2chunks = []
    for _ in range(NCH):
        q8 = np.clip(resid * S1, -FP8_MAX, FP8_MAX).astype(np.float32).astype(
            ml_dtypes.float8_e4m3)
        w2chunks.append(q8)
        resid = resid - q8.astype(np.float64) / S1
    Q = sum(c.astype(np.float64) for c in w2chunks) / S1  # reconstructed W2/2
    G1L = SCAN1_GROUPS[-1][0]
    w2e = np.ascontiguousarray(
        np.stack([c[:, :, :G1L, :] for c in w2chunks], axis=1))
    # late tiles padded with a zero k-tile (pairs with zero-spike tile 45)
    w2l = np.zeros((MT, NCH, 128, MT + 1 - G1L, 128), ml_dtypes.float8_e4m3)
    w2l[:, :, :, :MT - G1L, :] = np.stack(
        [c[:, :, G1L:, :] for c in w2chunks], axis=1)
    # GEMM3: hi/lo bf16 split on W3/2, stacked into one [128,2,MT,DOUT] arr
    W3t = fco_w.reshape(DOUT, MT, 128).transpose(2, 1, 0) * 0.5  # [p, k, q]
    w3hi = _bf(W3t)
    w3lo = _bf(W3t - w3hi.astype(np.float32))
    w3 = np.ascontiguousarray(np.stack([w3hi, w3lo], axis=1))
    # biases: threshold shift -(1-beta), plus the sign-encoding correction
    # +rowsum(W/2) of the reconstructed shipped chunks (fp64 for exactness)
    c2 = Q.sum(axis=(1, 2))  # [m, q]
    b2c = (fc2_b.astype(np.float64).reshape(MT, 128)
           - (1.0 - BETA) * THRESH + c2).astype(np.float32)
    b1s = (fc1_b - (1.0 - BETA) * THRESH).reshape(MT, 128).T
    b12 = np.ascontiguousarray(
        np.concatenate([b1s, b2c.T], axis=1).astype(np.float32))
    # GEMM3 host-side bias correction: rowsum of shipped (W3/2) split,
    # only over the on-chip-contracted tiles 0..MT3-1 (the host tail for
    # tiles 42-44 uses 0/1 spikes against the true fco_w — no correction)
    b3c = (w3hi.astype(np.float64)
           + w3lo.astype(np.float64))[:, :MT3, :].sum(axis=(0, 1))
    out = dict(
        inputs=dict(w1hi=w1hi, w1lo=w1lo, w2e=w2e, w2l=w2l,
                    w3=w3, b12=b12),
        b3c=b3c.astype(np.float32),
    )
    _prep_cache[key] = out
    return out


def _prep_x(x, T):
    """Per-core x arrays (r = t*BL + b), f16 hi + 2^12-scaled f16 lo.
    xhi [128, KP, R]; xlo [128, KP, R] with K-tile 18 packed as
    [xl8_s; xh8] to pair with w1lo's [wh8; wl8_s]."""
    xf = np.asarray(x, np.float32).reshape(B, T, -1)
    outs = []
    for c in range(NCORES):
        xc = xf[BL * c : BL * (c + 1)]            # [BL, T, DIN]
        xp = np.zeros((DINP, T * BL), np.float32)
        xp[:DIN] = xc.transpose(2, 1, 0).reshape(DIN, T * BL)
        xt = np.ascontiguousarray(xp.reshape(KP, 128, T * BL).transpose(1, 0, 2))
        xhi = xt.astype(np.float16)
        xlo = ((xt - xhi.astype(np.float32)) * 4096.0).astype(np.float16)
        xlo[8:16, KF, :] = xhi[0:8, KF, :]
        outs.append((np.ascontiguousarray(xhi), np.ascontiguousarray(xlo)))
    return outs


def kernel(x, fc1_w, fc1_b, fc2_w, fc2_b, fco_w, fco_b, _T=None, _want_results=False,
           _trace=False):
    T = _T or T_FULL
    if T not in _nc_cache:
        _nc_cache[T] = _build(T)
    nc = _nc_cache[T]

    w = _prep_weights(
        np.asarray(fc1_w, np.float32), np.asarray(fc1_b, np.float32),
        np.asarray(fc2_w, np.float32), np.asarray(fc2_b, np.float32),
        np.asarray(fco_w, np.float32),
    )
    xs = _prep_x(x, T)
    in_maps = [{"xhi": xs[c][0], "xlo": xs[c][1], **w["inputs"]}
               for c in range(NCORES)]
    res = run_bass_kernel_spmd(nc, in_maps, list(range(NCORES)), trace=_trace)

    # host: LIF scan + GEMM3 contribution of h2 tiles 42-44 (shipped raw),
    # then output-layer LIF scan + T-sum + pairwise voting (exact fp32)
    h2t = np.stack([res.results[c]["h2t"] for c in range(NCORES)])
    # [8, 128, 3, R]: feature = 128*(MT3+j)+p, r = t*BL+b (shifted domain:
    # bias includes -(1-beta)*THRESH, so threshold is 0 and init is -1)
    h2r = h2t.reshape(NCORES, 128, HOST_TILES, T, BL)
    mq = np.full((NCORES, 128, HOST_TILES, BL), -1.0, np.float32)
    s2t = np.empty((T, NCORES, 128, HOST_TILES, BL), np.float32)
    for t in range(T):
        mq = BETA * mq + h2r[:, :, :, t, :] - (mq > 0).astype(np.float32)
        s2t[t] = (mq > 0).astype(np.float32)
    # [T, c, p, j, b] -> [T, c, b, j*128+p]
    s2t = s2t.transpose(0, 1, 4, 3, 2).reshape(T, B, HOST_TILES * 128)
    w3tail = np.asarray(fco_w, np.float32)[:, MT3 * 128:]  # [90, 384]
    tail3 = s2t @ w3tail.T  # [T, B, 90]

    h3 = np.stack([res.results[c]["h3"] for c in range(NCORES)])  # [8, 90, R]
    i3 = h3.reshape(NCORES, DOUT, T, BL) \
        + (np.asarray(fco_b, np.float32) + w["b3c"])[None, :, None, None]
    i3 = i3.transpose(2, 0, 3, 1).reshape(T, B, DOUT) + tail3  # [T, 32, 90]
    m = np.zeros((B, DOUT), np.float32)
    s = np.zeros((B, DOUT), np.float32)
    out = np.zeros((B, DOUT), np.float32)
    for t in range(T):
        m = BETA * m + i3[t] - s * THRESH
        s = ((m - THRESH) > 0).astype(np.float32)
        out += s
    pi, pj = np.triu_indices(NUM_CLASSES, 1)
    outp = out.reshape(B, TRI_NUM, 2)
    votes = np.zeros((B, NUM_CLASSES), np.float32)
    np.add.at(votes, (slice(None), pi), outp[..., 0])
    np.add.at(votes, (slice(None), pj), outp[..., 1])
    if _want_results:
        return votes, res
    return votes



# revision 39
# speedup vs baseline: 1.0632x; 1.0632x over previous
"""TRN2 Bass kernel for nn_BSquareModelCombined (spiking MLP, LIF neurons).

Strategy
--------
The reference scans over T=100 steps, but the GEMMs are state-independent:
  h1 = x_t @ W1^T  for all t  -> one big GEMM over R = T*B_loc rows
  LIF scan (elementwise) -> spikes s1
  h2 = s1 @ W2^T   -> one big GEMM;  LIF scan -> s2
  h3 = s2 @ W3^T   -> small GEMM; output-layer scan + voting on host.

Data-parallel over batch: 8 cores x 4 batch rows. On-chip layout is
feature-major ("transposed"): activations are [D, R] with r = t*4+b, so the
GEMM moving operand is an activation tile [128, R=400] and the stationary
operand is a weight tile [128, 128].

Precision (the LIF thresholds make the network chaotic; host sims show the
final votes need ~16-bit weight fidelity in W1/W2 and ~24-bit x, while W3
tolerates 12-bit):
 - GEMM1: 3 passes in float32r (PE truncates operands to a 12-bit
   significand; a 12-bit hi/lo split of x and W1 is exactly representable,
   so xhi*Whi + xlo*Whi + xhi*Wlo is fp32-exact minus a 2^-24 term).
   All 3 passes accumulate into ONE PSUM group per m-tile (single
   eviction), and the last K-tile (only 8 real features of 2312) packs
   all three pass contributions into one matmul along spare partitions.
 - GEMM2: spikes are sign-encoded (g = sign(m) = 2s-1), stored as fp8.
   h = g @ (W/2)^T + rowsum(W/2), with W/2 decomposed into NCH=3 fp8e4m3
   chunks at one global scale S1=2^14 (clip+RNE greedy residual split;
   ~13-bit effective fidelity, host sims show final votes at ~1.3e-2 rel
   vs the 2e-2 gate). Both operands fp8 enables DoubleRow perf mode: one
   matmul contracts TWO k-tiles at bf16-rate (2x fp8 throughput measured
   on HW), so 3 chunks cost 69 matmul-slots/m-tile vs 90 for bf16 hi/lo.
   All chunks share one PSUM group; eviction applies 1/S1 and the bias
   (rowsum correction over the reconstructed chunks, fp64 on host).
 - GEMM3: spikes fp8 moving x bf16 stationary hi/lo (legal — only 32-bit
   dtypes must match).

LIF scan: one fused custom DVE op per step computes
   m_t = beta*m + h_t - (m > 0)   (reset recomputed from sign, not stored)
and the Scalar engine's Sign activation emits g_t = sign(m_t) off the
critical path. A serial chain steps at ~320ns (op + result-commit), so the
last group of each scan — whose input finalizes only when the producing
GEMM ends — trails by ~32us. Mitigations: small last groups, JIT K-order
in the consuming GEMM, and interleaved PSUM accumulation groups for
GEMM2's first two m-tiles to widen the overlap window.
"""
import sys

sys.path.insert(0, "/opt/trn_rl_repo")
sys.path.insert(0, "/root/.axon_site")

import numpy as np
import ml_dtypes

import concourse.bass as bass  # noqa: F401
import concourse.tile as tile
from concourse import bacc, mybir
from concourse import dve_ops
from concourse.dve_spec import Spec, Src0, Src1, C0, Zero, lower as dve_lower
from concourse.dve_uop import DveOpSpec
from concourse.bass_utils import run_bass_kernel_spmd

F32 = mybir.dt.float32
F32R = mybir.dt.float32r
F16 = mybir.dt.float16
BF16 = mybir.dt.bfloat16
FP8 = mybir.dt.float8e4

B, T_FULL, DIN, DH, DOUT = 32, 100, 2312, 5760, 90
NCORES = 8
BL = B // NCORES            # batch rows per core
KP = 19                     # D_in tiles after padding 2312 -> 2432
KF = 18                     # full 128-deep K tiles; tile 18 holds 8 feats
DINP = KP * 128
MT = DH // 128              # 45 feature tiles
BETA, THRESH = 0.9, 1.0
NUM_CLASSES, TRI_NUM = 10, 45
NCH = 3                     # fp8 chunks of W2/2 (4 = extra-safe fallback)
S1E = 13                    # chunk scale exponent: stored = fp8(W/2 * 2^S1E)
FP8_MAX = 240.0             # mybir float8e4 is IEEE e4m3: exp 1111 = inf/nan
DR = mybir.MatmulPerfMode.DoubleRow

_nc_cache = {}
_prep_cache = {}
DEBUG_TAPS = False          # extra DRAM outputs (g1, full h2) for debugging


def _register_lif_op():
    """Fused LIF membrane update: out = s0*in0 + in1 - (in0 > 0)."""
    name = "LIF_STEP_ANT"
    for o in dve_ops.OPS:
        if o.name == name:
            return o
    spec = Spec(
        body=(Src0 * C0) + Src1 - (Src0 > Zero),
        reference=lambda in0, in1, s0, s1, imm2: in0.astype(np.float32) * s0
        + in1.reshape(in0.shape)
        - (in0 > 0).astype(np.float32),
    )
    row = max(dve_ops._SUB_OPCODE_FOR_NAME.values()) + 1
    shas = {}
    for ver in ("v3", "v4"):
        uops = dve_lower(spec, ver=ver)
        shas[ver] = DveOpSpec(name=name, opcode=row, uops=uops, rd1_en=True).sha(ver)
    op = dve_ops.DveOp(name, spec, subdim=False, uops_sha=shas)
    dve_ops.OPS.append(op)
    dve_ops.CUSTOM_DVE_SPECS[name] = spec
    dve_ops._SUB_OPCODE_FOR_NAME[name] = row
    return op


LIF_OP = _register_lif_op()

# scan group layouts: (start_tile, n_tiles) lists. The last group is small
# so its 100-step serial DVE chain (the only part that can't hide under the
# producing GEMM) ends sooner; the consuming GEMM orders that group's
# K-tiles last (JIT) to hide the remaining chain latency.
SCAN1_GROUPS = [(0, 15), (15, 15), (30, 10), (40, 5)]
# scan2 covers only tiles 0..41 on-chip: tiles 42-44 evict LAST from GEMM2
# and their scan + GEMM3 contribution moves to the host (h2t output), so no
# scan chain ever trails the last GEMM — GEMM3 runs stall-free.
SCAN2_GROUPS = [(0, 9), (9, 9), (18, 9), (27, 9), (36, 6)]
MT3 = 42                    # feature tiles contracted on-chip in GEMM3
HOST_TILES = 3              # h2 tiles 42-44 handled on host


def _build(T):
    """Build + compile the per-core program (same program on all 8 cores)."""
    R = T * BL
    nc = bacc.Bacc(None, target_bir_lowering=False)

    # x split into per-K-tile chunks so the first matmul starts early
    # GEMM1 fully f16 (walrus only allows f32r paired with f32r): x and W1
    # each split as f16 hi + 2^12-scaled f16 lo (~22-bit effective). The
    # hi*hi pass accumulates in PSUM group A; both refinement passes
    # (xl_s@wh and xh@wl_s) carry the same 2^12 scale and share group B,
    # descaled at eviction. Halves both x and W1 DMA vs f32r.
    xhi_d = nc.dram_tensor("xhi", [128, KP, R], F16, kind="ExternalInput")
    xlo_d = nc.dram_tensor("xlo", [128, KP, R], F16, kind="ExternalInput")
    w1hi_d = nc.dram_tensor("w1hi", [MT, 128, KP, 128], F16, kind="ExternalInput")
    w1lo_d = nc.dram_tensor("w1lo", [MT, 128, KP, 128], F16, kind="ExternalInput")
    G1L = SCAN1_GROUPS[-1][0]   # deferral boundary (40, even: DR pairs align)
    # late tiles padded to an even 6 k-tiles (k45 = zeros, paired with the
    # zero-spike g1 tile 45) so every GEMM2 matmul is DoubleRow — a
    # DR<->normal perf-mode switch costs a ~310ns PE bubble.
    MTP = MT + 1
    w2e_d = nc.dram_tensor("w2e", [MT, NCH, 128, G1L, 128], FP8,
                           kind="ExternalInput")
    w2l_d = nc.dram_tensor("w2l", [MT, NCH, 128, MTP - G1L, 128], FP8,
                           kind="ExternalInput")
    w3_d = nc.dram_tensor("w3", [128, 2, MT, DOUT], BF16, kind="ExternalInput")
    b12_d = nc.dram_tensor("b12", [128, 2 * MT], F32, kind="ExternalInput")
    h3o = nc.dram_tensor("h3", [DOUT, R], F32, kind="ExternalOutput")
    h2t_d = nc.dram_tensor("h2t", [128, HOST_TILES, R], F32,
                           kind="ExternalOutput")

    add = mybir.AluOpType.add
    mult = mybir.AluOpType.mult

    with tile.TileContext(nc) as tc:
        with (
            tc.tile_pool(name="acts", bufs=1) as acts,
            tc.tile_pool(name="wpool", bufs=5) as wpool,
            tc.tile_pool(name="small", bufs=1) as small,
            tc.tile_pool(name="psum", bufs=4, space="PSUM") as pp,
        ):
            xhi = acts.tile([128, KP, R], F16, tag="xhi")
            xlo = acts.tile([128, KP, R], F16, tag="s_or_xlo")
            hsb = acts.tile([128, MT, R], F32, tag="h")
            b12sb = small.tile([128, 2 * MT], F32)
            # Rotating scan-state buffers shared by all groups of both
            # scans; each group uses its own disjoint column slice.
            # 6 buffers (not 3): the Scalar sign of step t reads buffer
            # (t+1)%NB, and with a short rotation the step t+NB-1 DVE write
            # would wait on that cross-engine read (~300ns sign) every
            # rotation, throttling the serial scan chain.
            NB = 5
            sts = [small.tile([128, MT, BL], F32, name=f"st{i}")
                   for i in range(NB)]

            # ---- GEMM1: group A = xh@wh (19 mm); group B (2^12 scale) =
            # xh@wl_s (18) + xl_s@wh (18) + packed k18 tail (1). Pass order
            # puts the xh-moving passes first so m0 runs 37 of its 56
            # matmuls while the xl stream is still landing.
            with nc.named_scope("gemm1"):
                for m in range(MT):
                    whi = wpool.tile([128, KP, 128], F16, tag="w")
                    wlo = wpool.tile([128, KP, 128], F16, tag="w")
                    if m == 0:
                        # startup: the first matmul needs only whi k0..3
                        # plus xhi k0, not the full whi tile. wlo and b12
                        # ride ahead of the x streams.
                        nc.sync.dma_start(out=whi[:, 0:4, :],
                                          in_=w1hi_d.ap()[m][:, 0:4])
                        nc.sync.dma_start(out=whi[:, 4:KP, :],
                                          in_=w1hi_d.ap()[m][:, 4:KP])
                        nc.sync.dma_start(out=b12sb[:], in_=b12_d.ap())
                        nc.sync.dma_start(out=wlo[:], in_=w1lo_d.ap()[m])
                        for k in range(KP):
                            nc.sync.dma_start(out=xhi[:, k, :],
                                              in_=xhi_d.ap()[:, k])
                        for k in range(KP):
                            nc.sync.dma_start(out=xlo[:, k, :],
                                              in_=xlo_d.ap()[:, k])
                    else:
                        nc.sync.dma_start(out=whi[:], in_=w1hi_d.ap()[m])
                        nc.sync.dma_start(out=wlo[:], in_=w1lo_d.ap()[m])
                    psa = pp.tile([128, R], F32, tag="ps", name=f"psa_{m}")
                    psb = pp.tile([128, R], F32, tag="ps", name=f"psb_{m}")
                    # pass 1: xh @ wh
                    for k in range(KP):
                        nc.tensor.matmul(
                            psa[:], whi[:, k, :], xhi[:, k, :],
                            start=(k == 0), stop=(k == KP - 1),
                        )
                    # pass 2: xh @ wl_s (group B)
                    for k in range(KF):
                        nc.tensor.matmul(
                            psb[:], wlo[:, k, :], xhi[:, k, :],
                            start=(k == 0), stop=False,
                        )
                    # pass 3: xl_s @ wh (group B)
                    for k in range(KF):
                        nc.tensor.matmul(
                            psb[:], whi[:, k, :], xlo[:, k, :],
                            start=False, stop=False,
                        )
                    # k18 tail, packed: moving [xl8_s; xh8] x [wh8; wl8_s]
                    nc.tensor.matmul(
                        psb[:], wlo[:, KF, :], xlo[:, KF, :],
                        start=False, stop=True,
                    )
                    nc.vector.tensor_scalar(
                        hsb[:, m, :], psb[:], float(2.0 ** -12),
                        b12sb[:, m : m + 1], mult, add
                    )
                    nc.vector.tensor_tensor(
                        hsb[:, m, :], hsb[:, m, :], psa[:], add
                    )

            # spikes, sign-encoded (+1 spike / -1 no spike), fp8 (exact).
            # g1 gets its OWN region (no alias with x): its signs must run
            # during GEMM1's tail, while xhi/xlo are still being read.
            # g1 tile 45 is the zero pad partnering w2l's k45 zero weights.
            g1sb = acts.tile([128, MTP, R], FP8, tag="g1")
            nc.vector.memset(g1sb[:, MT, :], 0.0)
            g2sb = acts.tile([128, MT, R], FP8, tag="xhi")

            def lif_scan(scope, gsb, groups):
                # chunk-group scans: group g only depends on its own feature
                # tiles, so it starts as soon as the producing GEMM has
                # evicted those tiles and hides under the GEMM. The LAST
                # group's chain cannot hide (its inputs finalize only when
                # the GEMM ends), and a single chain steps at ~320ns (op +
                # result-commit latency). Splitting it into BL independent
                # per-batch-column chains interleaves 4 ops per step, hiding
                # the commit latency — identical math, ~2x lower latency.
                with nc.named_scope(scope):
                    for g, (c0, n) in enumerate(groups):
                        c1 = c0 + n
                        nc.vector.memset(sts[0][:, c0:c1, :], -1.0)
                        for t in range(T):
                            hsl = hsb[:, c0:c1, BL * t : BL * (t + 1)]
                            gsl = gsb[:, c0:c1, BL * t : BL * (t + 1)]
                            src = sts[t % NB][:, c0:c1, :]
                            dst = sts[(t + 1) % NB][:, c0:c1, :]
                            nc.vector._custom_dve(
                                LIF_OP, out=dst, in0=src, in1=hsl, s0=BETA
                            )
                            nc.scalar.sign(gsl, dst)

            lif_scan("scan1", g1sb, SCAN1_GROUPS)

            # ---- GEMM2: h2 = g1 @ (W2/2)^T + bias'' (NCH fp8 chunks) ----
            # The last scan1 group's (spike tiles 40-44) serial chain only
            # starts after GEMM1's final eviction. To hide its ~32us, the
            # first HEAD_M output tiles emit only k<G1L (all chunks) as
            # open PSUM groups, then their k>=G1L tail runs. Weights are
            # split early/late at G1L so a deferred m-tile pins only its
            # tiny late tiles. Output tiles 42-44 (host-bound) come last.
            M_ORDER = list(range(MT3)) + list(range(MT3, MT))
            HEAD_M = 4
            TOTAL_MM = NCH * (G1L // 2 + (MTP - G1L) // 2)

            def g2_weights(m):
                wes, wls = [], []
                for c in range(NCH):
                    we = wpool.tile([128, G1L, 128], FP8, tag="w2e",
                                    bufs=6, name=f"w2e_{m}_{c}")
                    nc.sync.dma_start(out=we[:], in_=w2e_d.ap()[m][c])
                    wes.append(we)
                for c in range(NCH):
                    wl = wpool.tile([128, MTP - G1L, 128], FP8, tag="w2l",
                                    bufs=3 * HEAD_M, name=f"w2l_{m}_{c}")
                    nc.sync.dma_start(out=wl[:], in_=w2l_d.ap()[m][c])
                    wls.append(wl)
                return wes, wls

            def g2_evict(m, ps):
                nc.vector.tensor_scalar(
                    hsb[:, m, :], ps[:], float(2.0 ** -S1E),
                    b12sb[:, MT + m : MT + m + 1], mult, add
                )

            with nc.named_scope("gemm2"):
                st = {}  # m -> [ps, wes, wls, nmm]

                def emit(m, lo, hi):
                    # hi == MT means "through the padded tile 45"
                    hi_pad = MTP if hi == MT else hi
                    ps, wes, wls, nmm = st[m]
                    for c in range(NCH):
                        for k in range(lo, hi_pad, 2):
                            w_ap = (wes[c][:, k : k + 2, :] if k < G1L
                                    else wls[c][:, k - G1L : k - G1L + 2, :])
                            nc.tensor.matmul(
                                ps[:], w_ap, g1sb[:, k : k + 2, :],
                                start=(nmm == 0),
                                stop=(nmm == TOTAL_MM - 1),
                                perf_mode=DR,
                            )
                            nmm += 1
                    st[m][3] = nmm

                for m in M_ORDER[:HEAD_M]:
                    wes, wls = g2_weights(m)
                    ps = pp.tile([128, R], F32, tag="ps", name=f"ps_g2_{m}")
                    st[m] = [ps, wes, wls, 0]
                    emit(m, 0, G1L)
                for m in M_ORDER[:HEAD_M]:
                    emit(m, G1L, MT)
                    g2_evict(m, st[m][0])
                # steady state
                w3sb = None
                for i, m in enumerate(M_ORDER[HEAD_M:]):
                    if i == len(M_ORDER) - HEAD_M - 3:
                        # prefetch W3 (hi+lo halves in one tile, living in
                        # the long-dead xlo region) so GEMM3 starts clean
                        w3sb = acts.tile([128, 2, MT, DOUT], BF16,
                                         tag="s_or_xlo", name="w3sb")
                        nc.sync.dma_start(out=w3sb[:], in_=w3_d.ap())
                    wes, wls = g2_weights(m)
                    ps = pp.tile([128, R], F32, tag="ps", name=f"ps_g2_{m}")
                    st[m] = [ps, wes, wls, 0]
                    emit(m, 0, MT)
                    g2_evict(m, ps)
                # ship host-bound h2 tiles (42-44) for the host-side tail
                nc.sync.dma_start(out=h2t_d.ap(), in_=hsb[:, MT3:MT, :])

            if DEBUG_TAPS:
                h2f_d = nc.dram_tensor("h2f", [128, MT, R], F32,
                                       kind="ExternalOutput")
                nc.sync.dma_start(out=h2f_d.ap(), in_=hsb[:])
                g1o_d = nc.dram_tensor("g1o", [128, MT, R], FP8,
                                       kind="ExternalOutput")
                nc.sync.dma_start(out=g1o_d.ap(), in_=g1sb[:, :MT, :])

            lif_scan("scan2", g2sb, SCAN2_GROUPS)

            if DEBUG_TAPS:
                g2o_d = nc.dram_tensor("g2o", [128, MT3, R], FP8,
                                       kind="ExternalOutput")
                nc.sync.dma_start(out=g2o_d.ap(), in_=g2sb[:, :MT3, :])

            # ---- GEMM3: h3 = g2 @ (W3/2)^T (hi/lo bf16), out [90, R] ----
            # only tiles 0..41: every scan2 group finishes before GEMM2
            # does (the last group's tiles evict 3 m-tiles early), so these
            # 84 matmuls run without any scan stall. Tiles 42-44 are summed
            # on the host from the h2t output.
            with nc.named_scope("gemm3"):
                ps3 = pp.tile([DOUT, R], F32, tag="ps3")
                nmm = 0
                for k in range(MT3):
                    for h in (0, 1):
                        nc.tensor.matmul(
                            ps3[:], w3sb[:, h, k, :], g2sb[:, k, :],
                            start=(nmm == 0), stop=(nmm == 2 * MT3 - 1),
                        )
                        nmm += 1
                h3sb = small.tile([DOUT, R], F32, tag="h3sb")
                nc.vector.tensor_copy(h3sb[:], ps3[:])
                nc.sync.dma_start(out=h3o.ap(), in_=h3sb[:])

    nc.compile()
    return nc


def _bf(a):
    return a.astype(ml_dtypes.bfloat16)


def _round12(a):
    """Round fp32 to 12-bit significand (11 explicit mantissa bits), RNE —
    the f32r PE operand grid; representable values pass the PE unchanged."""
    u = np.ascontiguousarray(a, np.float32).view(np.uint32)
    u = (u + 0x7FF + ((u >> 12) & 1)) & np.uint32(0xFFFFF000)
    return u.view(np.float32)


def _prep_weights(fc1_w, fc1_b, fc2_w, fc2_b, fco_w):
    key = (fc1_w.ctypes.data, fc2_w.ctypes.data, fco_w.ctypes.data)
    if key in _prep_cache:
        return _prep_cache[key]
    # GEMM1: f16 hi + 2^12-scaled f16 lo split of W1^T tiles (~22-bit W1)
    W1p = np.zeros((DH, DINP), np.float32)
    W1p[:, :DIN] = fc1_w
    W1t = np.ascontiguousarray(
        W1p.reshape(MT, 128, KP, 128).transpose(0, 3, 2, 1)
    )  # [m, p, k, q] = W1[m*128+q, k*128+p]
    w1hi = W1t.astype(np.float16)
    w1lo = ((W1t - w1hi.astype(np.float32)) * 4096.0).astype(np.float16)
    # K-tile 18 (8 real features): group A's matmul uses w1hi k18 p0-7 as
    # is. Group B's single packed k18 matmul uses the w1lo tile laid out
    # [wh8; wl8_s] against moving [xl8_s; xh8].
    lo18 = w1lo[:, 0:8, KF, :].copy()
    w1lo[:, 0:8, KF, :] = w1hi[:, 0:8, KF, :]
    w1lo[:, 8:16, KF, :] = lo18
    w1hi = np.ascontiguousarray(w1hi)
    w1lo = np.ascontiguousarray(w1lo)
    # GEMM2: sign-encoded spikes -> weights W2/2, NCH fp8e4m3 chunks at one
    # global scale 2^S1E (greedy clip+RNE residual split; stored = r*2^S1E)
    W2t = fc2_w.reshape(MT, 128, MT, 128).transpose(0, 3, 2, 1) * 0.5
    S1 = float(2.0 ** S1E)
    resid = W2t.astype(np.float64)
    w2chunks = []
    for _ in range(NCH):
        q8 = np.clip(resid * S1, -FP8_MAX, FP8_MAX).astype(np.float32).astype(
            ml_dtypes.float8_e4m3)
        w2chunks.append(q8)
        resid = resid - q8.astype(np.float64) / S1
    Q = sum(c.astype(np.float64) for c in w2chunks) / S1  # reconstructed W2/2
    G1L = SCAN1_GROUPS[-1][0]
    w2e = np.ascontiguousarray(
        np.stack([c[:, :, :G1L, :] for c in w2chunks], axis=1))
    # late tiles padded with a zero k-tile (pairs with zero-spike tile 45)
    w2l = np.zeros((MT, NCH, 128, MT + 1 - G1L, 128), ml_dtypes.float8_e4m3)
    w2l[:, :, :, :MT - G1L, :] = np.stack(
        [c[:, :, G1L:, :] for c in w2chunks], axis=1)
    # GEMM3: hi/lo bf16 split on W3/2, stacked into one [128,2,MT,DOUT] arr
    W3t = fco_w.reshape(DOUT, MT, 128).transpose(2, 1, 0) * 0.5  # [p, k, q]
    w3hi = _bf(W3t)
    w3lo = _bf(W3t - w3hi.astype(np.float32))
    w3 = np.ascontiguousarray(np.stack([w3hi, w3lo], axis=1))
    # biases: threshold shift -(1-beta), plus the sign-encoding correction
    # +rowsum(W/2) of the reconstructed shipped chunks (fp64 for exactness)
    c2 = Q.sum(axis=(1, 2))  # [m, q]
    b2c = (fc2_b.astype(np.float64).reshape(MT, 128)
           - (1.0 - BETA) * THRESH + c2).astype(np.float32)
    b1s = (fc1_b - (1.0 - BETA) * THRESH).reshape(MT, 128).T
    b12 = np.ascontiguousarray(
        np.concatenate([b1s, b2c.T], axis=1).astype(np.float32))
    # GEMM3 host-side bias correction: rowsum of shipped (W3/2) split,
    # only over the on-chip-contracted tiles 0..MT3-1 (the host tail for
    # tiles 42-44 uses 0/1 spikes against the true fco_w — no correction)
    b3c = (w3hi.astype(np.float64)
           + w3lo.astype(np.float64))[:, :MT3, :].sum(axis=(0, 1))
    out = dict(
        inputs=dict(w1hi=w1hi, w1lo=w1lo, w2e=w2e, w2l=w2l,
                    w3=w3, b12=b12),
        b3c=b3c.astype(np.float32),
    )
    _prep_cache[key] = out
    return out


def _prep_x(x, T):
    """Per-core x arrays (r = t*BL + b), f16 hi + 2^12-scaled f16 lo.
    xhi [128, KP, R]; xlo [128, KP, R] with K-tile 18 packed as
    [xl8_s; xh8] to pair with w1lo's [wh8; wl8_s]."""
    xf = np.asarray(x, np.float32).reshape(B, T, -1)
    outs = []
    for c in range(NCORES):
        xc = xf[BL * c : BL * (c + 1)]            # [BL, T, DIN]
        xp = np.zeros((DINP, T * BL), np.float32)
        xp[:DIN] = xc.transpose(2, 1, 0).reshape(DIN, T * BL)
        xt = np.ascontiguousarray(xp.reshape(KP, 128, T * BL).transpose(1, 0, 2))
        xhi = xt.astype(np.float16)
        xlo = ((xt - xhi.astype(np.float32)) * 4096.0).astype(np.float16)
        xlo[8:16, KF, :] = xhi[0:8, KF, :]
        outs.append((np.ascontiguousarray(xhi), np.ascontiguousarray(xlo)))
    return outs


def kernel(x, fc1_w, fc1_b, fc2_w, fc2_b, fco_w, fco_b, _T=None, _want_results=False,
           _trace=False):
    T = _T or T_FULL
    if T not in _nc_cache:
        _nc_cache[T] = _build(T)
    nc = _nc_cache[T]

    w = _prep_weights(
        np.asarray(fc1_w, np.float32), np.asarray(fc1_b, np.float32),
        np.asarray(fc2_w, np.float32), np.asarray(fc2_b, np.float32),
        np.asarray(fco_w, np.float32),
    )
    xs = _prep_x(x, T)
    in_maps = [{"xhi": xs[c][0], "xlo": xs[c][1], **w["inputs"]}
               for c in range(NCORES)]
    res = run_bass_kernel_spmd(nc, in_maps, list(range(NCORES)), trace=_trace)

    # host: LIF scan + GEMM3 contribution of h2 tiles 42-44 (shipped raw),
    # then output-layer LIF scan + T-sum + pairwise voting (exact fp32)
    h2t = np.stack([res.results[c]["h2t"] for c in range(NCORES)])
    # [8, 128, 3, R]: feature = 128*(MT3+j)+p, r = t*BL+b (shifted domain:
    # bias includes -(1-beta)*THRESH, so threshold is 0 and init is -1)
    h2r = h2t.reshape(NCORES, 128, HOST_TILES, T, BL)
    mq = np.full((NCORES, 128, HOST_TILES, BL), -1.0, np.float32)
    s2t = np.empty((T, NCORES, 128, HOST_TILES, BL), np.float32)
    for t in range(T):
        mq = BETA * mq + h2r[:, :, :, t, :] - (mq > 0).astype(np.float32)
        s2t[t] = (mq > 0).astype(np.float32)
    # [T, c, p, j, b] -> [T, c, b, j*128+p]
    s2t = s2t.transpose(0, 1, 4, 3, 2).reshape(T, B, HOST_TILES * 128)
    w3tail = np.asarray(fco_w, np.float32)[:, MT3 * 128:]  # [90, 384]
    tail3 = s2t @ w3tail.T  # [T, B, 90]

    h3 = np.stack([res.results[c]["h3"] for c in range(NCORES)])  # [8, 90, R]
    i3 = h3.reshape(NCORES, DOUT, T, BL) \
        + (np.asarray(fco_b, np.float32) + w["b3c"])[None, :, None, None]
    i3 = i3.transpose(2, 0, 3, 1).reshape(T, B, DOUT) + tail3  # [T, 32, 90]
    m = np.zeros((B, DOUT), np.float32)
    s = np.zeros((B, DOUT), np.float32)
    out = np.zeros((B, DOUT), np.float32)
    for t in range(T):
        m = BETA * m + i3[t] - s * THRESH
        s = ((m - THRESH) > 0).astype(np.float32)
        out += s
    pi, pj = np.triu_indices(NUM_CLASSES, 1)
    outp = out.reshape(B, TRI_NUM, 2)
    votes = np.zeros((B, NUM_CLASSES), np.float32)
    np.add.at(votes, (slice(None), pi), outp[..., 0])
    np.add.at(votes, (slice(None), pj), outp[..., 1])
    if _want_results:
        return votes, res
    return votes

